# revision 21
# baseline (speedup 1.0000x reference)
"""DDiT block (AdaLN-modulated transformer block) on 8 Trainium2 NeuronCores.

Sharding: pure data-parallel, core = (batch b in {0,1}) x (query-chunk k in
0..3 of 512 tokens).  Each core computes LN1/K/V over the full 2048-token
batch (K/V replicated within the 4 cores of a batch -- a collective exchange
at ~50GB/s would cost more than the 40us of redundant PE work), then
attention / out-proj / LN2 / MLP for its own 512 queries.  The host permutes
each core's tokens so its 512 queries are tokens 0:512 (attention over keys
is order-invariant) -> identical SPMD program on all cores.

All AdaLN modulation is folded out of the device elementwise path:
  x_mod = ((x-mu)*rstd)*a + sh   with a = (1+scale)*ln_w
  W @ x_mod = (W*diag(a)) @ xhat + (W @ sh)
so the host pre-scales every projection weight by its `a` vector, pre-gates
w_out/w_mlp2 by the AdaLN gates, and ships per-output-block bias columns
(W@sh).  The device only computes xhat = x*rstd + (-mu*rstd):
  - LN sums via ones-column matmuls (s1 at psum partition 0, s2 at
    partition 32 -> disjoint PE column-groups, the two 6-step chains overlap),
  - rstd = exp(-0.5*ln(var+eps)) on ScalarE (stays in the natural_log_exp
    ACT table set shared with attention's exp -> no table switching),
  - rstd/negmr broadcast across partitions by a ones-row PE matmul (no
    DRAM bounce),
  - xhat applied in place on the bf16 x tiles with two DVE tensor_tensor ops.
The V-projection shift contributes a constant per-d vector after softmax
(weights sum to 1), so it is folded host-side into the residual input.

Softmax: scores for 3 key-tiles land in one [128,1536] PSUM tile (a/b
alternated), one Exp ACT per group; denominator comes free from a ones
column appended to V (AV matmul row 64).  Normalization is deferred: the
denominator rows are staged to SBUF and batch-reciprocal'd on DVE ([12,512]
costs the same as [1,512]), then broadcast via PE and multiplied in.

prec="bf16" everywhere on the GEMM path; LN stats, softmax accumulation and
residuals stay fp32.
"""

import contextlib

import numpy as np

import concourse.bass as bass
import concourse.mybir as mybir
import concourse.tile as tile
from concourse.bass_utils import run_bass_kernel_spmd

F32 = mybir.dt.float32
F32R = mybir.dt.float32r
BF16 = mybir.dt.bfloat16
FP8 = mybir.dt.float8e4
DR = mybir.MatmulPerfMode.DoubleRow
W8SCALE = 32.0  # qkv weights shipped x32 in fp8 (avoid subnormals), undone
                # by the free ACT scale on the PSUM-copy out
AF = mybir.ActivationFunctionType
OP = mybir.AluOpType

D = 768
S = 2048
H = 12
DH = 64
DC = D // 128           # 6 chunks of d on partitions
HID = 4 * D             # 3072
HC = HID // 128         # 24
NQ = 512                # queries per core
NCH = S // NQ           # 4 token chunks
NTP = S // 128          # 16 key tiles of 128
EPS = 1e-5


def _gelu_store(nc, vtmp, ps, bcol, out, mode):
    """out = gelu_tanh(ps + bcol), ps in PSUM.  mode 'fused' uses the ACT
    Gelu_apprx_tanh table; 'tanh' decomposes it exactly (CoreSim has no gelu).
    """
    if mode == "fused":
        nc.scalar.activation(out, ps, AF.Gelu_apprx_tanh, bias=bcol, scale=1.0)
        return
    xb = vtmp.tile([128, NQ], F32, tag="vtmp", bufs=4)
    nc.vector.tensor_scalar_add(xb, ps, bcol)
    x2 = vtmp.tile([128, NQ], F32, tag="vtmp", bufs=4)
    nc.vector.tensor_mul(x2, xb, xb)
    poly = vtmp.tile([128, NQ], F32, tag="vtmp", bufs=4)
    nc.vector.tensor_scalar(poly, x2, 0.044715, 1.0, op0=OP.mult, op1=OP.add)
    arg = vtmp.tile([128, NQ], F32, tag="vtmp", bufs=4)
    nc.vector.scalar_tensor_tensor(arg, in0=xb, scalar=0.7978845608028654,
                                   in1=poly, op0=OP.mult, op1=OP.mult)
    th = vtmp.tile([128, NQ], F32, tag="vtmp", bufs=4)
    nc.scalar.activation(th, arg, AF.Tanh)
    halfx = vtmp.tile([128, NQ], F32, tag="vtmp", bufs=4)
    nc.vector.tensor_scalar_mul(halfx, xb, 0.5)
    nc.vector.scalar_tensor_tensor(out, in0=th, scalar=1.0, in1=halfx,
                                   op0=OP.add, op1=OP.mult)


def _bcast_row(nc, dbc, dst, row):
    """Broadcast a [1, N] SBUF row across partitions of dst via a DRAM bounce
    (partition-step-0 source AP on the read back)."""
    scratch = dbc.tile([1, row.shape[-1]], row.dtype, tag="bc", bufs=4)
    nc.sync.dma_start(scratch, row)
    src = bass.AP(tensor=scratch.tensor, offset=scratch.offset,
                  ap=[[0, dst.shape[0]]] + list(scratch.ap[1:]))
    nc.sync.dma_start(dst, src)


def _ln_rows(nc, rows, s1, s2, ln_mode):
    """From PSUM sums s1=sum_d x, s2=sum_d x^2 ([1, NQ]) produce SBUF rows
    rstd[t] = 1/sqrt(var+eps) and negmr[t] = -mu[t]*rstd[t] (both F32R)."""
    mu = rows.tile([1, NQ], F32R, tag="rA")
    nc.vector.tensor_scalar_mul(mu, s1, 1.0 / D)
    ex2 = rows.tile([1, NQ], F32R, tag="rB")
    nc.vector.tensor_scalar_mul(ex2, s2, 1.0 / D)
    musq = rows.tile([1, NQ], F32R, tag="rC")
    nc.vector.tensor_mul(musq, mu, mu)
    var = rows.tile([1, NQ], F32R, tag="rD")
    nc.vector.scalar_tensor_tensor(var, in0=ex2, scalar=EPS, in1=musq,
                                   op0=OP.add, op1=OP.subtract)
    if ln_mode == "lnexp":
        # rstd = exp(-0.5*ln(var)) -- stays in the natural_log_exp ACT set.
        lnv = rows.tile([1, NQ], F32R, tag="rC")
        nc.scalar.activation(lnv, var, AF.Ln)
        rstd = rows.tile([1, NQ], F32R, tag="rB")
        nc.scalar.activation(rstd, lnv, AF.Exp, bias=0.0, scale=-0.5)
    else:
        # CoreSim-safe fallback: DVE reciprocal + ACT sqrt.
        rvar = rows.tile([1, NQ], F32R, tag="rC")
        nc.vector.reciprocal(rvar, var)
        rstd = rows.tile([1, NQ], F32R, tag="rB")
        nc.scalar.sqrt(rstd, rvar)
    negmr = rows.tile([1, NQ], F32R, tag="rD")
    nc.vector.scalar_tensor_tensor(negmr, in0=mu, scalar=-1.0, in1=rstd,
                                   op0=OP.mult, op1=OP.mult)
    return rstd, negmr


def _body(tc, dram, gelu_mode, ln_mode):
    nc = tc.nc
    r128 = lambda name: dram[name].ap().rearrange("(o p) j -> p o j", p=128)
    xT_r = r128("xT")           # [128, 6, 2048] bf16
    xskip_r = r128("xskipT")    # [128, 6, 512]  f32r
    wqkv_r = r128("w_qkvT")     # [128, 6, 2304] bf16
    wout_r = r128("w_outT")     # [128, 6, 768]  bf16
    wm1_r = r128("w_m1T")       # [128, 6, 3072] bf16
    wm2_r = r128("w_m2T")       # [128, 24, 768] bf16
    outT_r = r128("outT")       # [128, 6, 512]  f32r

    with contextlib.ExitStack() as ctx:
        main = ctx.enter_context(tc.tile_pool(name="main", bufs=1))
        wm1p = ctx.enter_context(tc.tile_pool(name="wm1p", bufs=1))
        rows = ctx.enter_context(tc.tile_pool(name="rows", bufs=1))
        dbc = ctx.enter_context(tc.tile_pool(name="dbc", bufs=4, space="DRAM"))

        BIG = dict(tag="big", bufs=4)    # x/xs chunks, later reused by hT
        KTR = dict(tag="ktr", bufs=4)    # kT chunks, later reused by w_m2
        WQ = dict(tag="w", bufs=4)       # wk/wq/wv ring; wout reuses
        VT = dict(tag="vt", bufs=8)      # v tile pairs (all 8 live)
        SQ = dict(tag="sq", bufs=3)      # x^2 scratch
        RBSB = dict(tag="rbsb", bufs=3)  # rstd/negmr broadcast in SBUF bf16
        ET = dict(tag="et", bufs=4)      # exp tiles
        DST = dict(tag="dst", bufs=2)    # denominator staging rows

        # constants / modulation columns (ones shipped from host: walrus
        # rejects memset on single-partition tiles)
        ones_row = main.tile([1, 128], F32R)
        nc.sync.dma_start(ones_row, dram["ones_r"].ap())
        cols = main.tile([128, 44], F32)
        nc.sync.dma_start(cols, dram["cols_c"].ap())
        ones_bf = main.tile([128, 1], BF16)
        nc.vector.tensor_copy(ones_bf, cols[:, 42:43])
        ones_f8 = main.tile([128, 1], FP8)
        nc.vector.tensor_copy(ones_f8, cols[:, 42:43])
        neg2_c = cols[:, 43:44]
        cq_c = cols[:, 0:6]
        ck_c = cols[:, 6:12]
        cm1_c = cols[:, 12:36]
        gb2_c = cols[:, 36:42]

        # x chunks (fp8; modulated in place into xhat)
        x_tiles = []
        for ch in range(NCH):
            x_t = main.tile([128, DC, NQ], FP8, name=f"x{ch}", **BIG)
            nc.sync.dma_start(x_t, xT_r[:, :, ch * NQ:(ch + 1) * NQ])
            x_tiles.append(x_t)

        # K weights (both halves prefetched)
        wk = [main.tile([128, DC, 384], FP8, name=f"wk{i}", **WQ)
              for i in range(2)]
        for half in range(2):
            nc.sync.dma_start(
                wk[half], wqkv_r[:, :, D + half * 384:D + (half + 1) * 384])

        # MLP1 weights: w1_0..3 prefetched early (overlaps attention);
        # w1_4/w1_5 ride the ktr ring (kT slots freed at attention end).
        w1_tiles = [wm1p.tile([128, DC, NQ], BF16, name=f"w1_{i}")
                    for i in range(4)]
        for wt in range(4):
            nc.sync.dma_start(w1_tiles[wt], wm1_r[:, :, wt * 512:(wt + 1) * 512])

        # residual input for my 512 queries (V-shift const folded in on host)
        xskip = main.tile([128, DC, NQ], F32R)
        nc.sync.dma_start(xskip, xskip_r)

        # ---- phase 1: LN stats + xhat + K/Q/V projections
        ph_stats = tc.tile_pool(name="pstats", bufs=2, space="PSUM")
        ph_rbmb = tc.tile_pool(name="prbmb", bufs=3, space="PSUM")
        ph_mm = tc.tile_pool(name="pmm", bufs=3, space="PSUM")
        stats = ph_stats.__enter__()
        rbmb = ph_rbmb.__enter__()
        psmm = ph_mm.__enter__()

        def stats_chunk(ch):
            ps_s = stats.tile([33, NQ], F32, tag="st")
            sqs = []
            for o in range(DC):
                sq = main.tile([128, NQ], BF16, **SQ)
                nc.vector.tensor_mul(sq, x_tiles[ch][:, o, :],
                                     x_tiles[ch][:, o, :])
                sqs.append(sq)
            for o in range(DC):
                nc.tensor.matmul(ps_s[0:1, :], ones_f8, x_tiles[ch][:, o, :],
                                 start=(o == 0), stop=(o == DC - 1))
                nc.tensor.matmul(ps_s[32:33, :], ones_bf, sqs[o],
                                 start=(o == 0), stop=(o == DC - 1))
            return ps_s

        def xhat_chunk(ch, ps_s):
            rstd, negmr = _ln_rows(nc, rows, ps_s[0:1, :], ps_s[32:33, :],
                                   ln_mode)
            rb_ps = rbmb.tile([128, NQ], F32, tag="bc")
            nc.tensor.matmul(rb_ps, ones_row, rstd, start=True, stop=True)
            mb_ps = rbmb.tile([128, NQ], F32, tag="bc")
            nc.tensor.matmul(mb_ps, ones_row, negmr, start=True, stop=True)
            rb_sb = main.tile([128, NQ], BF16, **RBSB)
            nc.vector.tensor_copy(rb_sb, rb_ps)
            mb_sb = main.tile([128, NQ], BF16, **RBSB)
            nc.vector.tensor_copy(mb_sb, mb_ps)
            x_t = x_tiles[ch]
            for o in range(DC):
                nc.vector.tensor_mul(x_t[:, o, :], x_t[:, o, :], rb_sb)
                nc.vector.tensor_add(x_t[:, o, :], x_t[:, o, :], mb_sb)

        k_tiles = []

        def k_chunk(ch):
            kt = main.tile([128, DC, NQ], BF16, name=f"kT{ch}", **KTR)
            for half in range(2):
                for sub in range(3):
                    mo = half * 3 + sub
                    ps = psmm.tile([128, NQ], F32, tag="mm")
                    for p in range(3):
                        nc.tensor.matmul(
                            ps,
                            wk[half][:, 2 * p:2 * p + 2,
                                     sub * 128:(sub + 1) * 128],
                            x_tiles[ch][:, 2 * p:2 * p + 2, :],
                            start=(p == 0), stop=(p == 2), perf_mode=DR)
                    nc.scalar.activation(kt[:, mo, :], ps, AF.Identity,
                                         bias=ck_c[:, mo:mo + 1],
                                         scale=1.0 / W8SCALE)
            k_tiles.append(kt)

        ps_stats = [stats_chunk(0), stats_chunk(1)]
        xhat_chunk(0, ps_stats[0])
        ps_stats.append(stats_chunk(2))
        k_chunk(0)

        # Q (my 512 queries; needs only xhat chunk 0)
        qT = main.tile([128, DC, NQ], BF16)
        wq = [main.tile([128, DC, 384], FP8, name=f"wq{i}", **WQ)
              for i in range(2)]
        for half in range(2):
            nc.sync.dma_start(wq[half],
                              wqkv_r[:, :, half * 384:(half + 1) * 384])
            for sub in range(3):
                mo = half * 3 + sub
                ps = psmm.tile([128, NQ], F32, tag="mm")
                for p in range(3):
                    nc.tensor.matmul(
                        ps,
                        wq[half][:, 2 * p:2 * p + 2,
                                 sub * 128:(sub + 1) * 128],
                        x_tiles[0][:, 2 * p:2 * p + 2, :],
                        start=(p == 0), stop=(p == 2), perf_mode=DR)
                nc.scalar.activation(qT[:, mo, :], ps, AF.Identity,
                                     bias=cq_c[:, mo:mo + 1],
                                     scale=1.0 / W8SCALE)

        xhat_chunk(1, ps_stats[1])
        ps_stats.append(stats_chunk(3))
        k_chunk(1)
        xhat_chunk(2, ps_stats[2])
        k_chunk(2)
        xhat_chunk(3, ps_stats[3])
        k_chunk(3)

        # V (natural layout, fp8, key-tiles paired for DoubleRow AV;
        # ones column in row 64 gives the softmax denominator)
        v_tiles = []
        for half in range(2):
            wv = main.tile([128, DC, 384], FP8, name=f"wv{half}", **WQ)
            nc.sync.dma_start(
                wv, wqkv_r[:, :, 2 * D + half * 384:2 * D + (half + 1) * 384])
            for tp in range(NTP):
                ch, sub = tp // 4, tp % 4
                if half == 0:
                    if tp % 2 == 0:
                        # inner dim padded 65->68 so the pair-step (12*68)
                        # is 16-aligned (DoubleRow LDWEIGHTS constraint)
                        vt = main.tile([128, 2, H, DH + 4], FP8,
                                       name=f"v{tp // 2}", **VT)
                        nc.vector.tensor_copy(
                            vt[:, :, :, DH:DH + 1],
                            ones_bf.to_broadcast((128, 2, H, 1)))
                        v_tiles.append(vt)
                vt = v_tiles[tp // 2]
                ps = psmm.tile([128, NQ], F32, tag="mm")
                for p in range(3):
                    nc.tensor.matmul(
                        ps[:, 0:384],
                        x_tiles[ch][:, 2 * p:2 * p + 2,
                                    sub * 128:(sub + 1) * 128],
                        wv[:, 2 * p:2 * p + 2, :],
                        start=(p == 0), stop=(p == 2), perf_mode=DR)
                nc.scalar.mul(
                    vt[:, tp % 2, half * 6:(half + 1) * 6, 0:DH],
                    ps[:, 0:384].rearrange("p (h d) -> p h d", h=6),
                    1.0 / W8SCALE)

        ph_mm.__exit__(None, None, None)
        ph_rbmb.__exit__(None, None, None)
        ph_stats.__exit__(None, None, None)

        # ---- attention: 12 heads in pairs (h, h+1).  The two heads of a
        # pair live on disjoint 64-row PE groups (pb 0 / 64), so their
        # interleaved scores matmuls run concurrently on disjoint sub-arrays
        # and each LDWEIGHTS pulls ahead under the other head's matmul.
        # Scores for 3 key-tiles land in one [128,1536] PSUM tile per head
        # (a/b pools double-buffer across groups), one Exp ACT per tile; AV
        # accumulates [65, NQ] with the denominator in row 64 (ones column
        # of V).  Normalization is deferred: denominator rows staged out,
        # batch-reciprocal'd on DVE, broadcast by DRAM bounce (no PSUM), and
        # multiplied in while later pairs still run.
        oT = main.tile([128, DC, NQ], BF16)
        # heads 0-7 at partitions 0-7, heads 8-11 at 32-35 (DVE base-partition
        # alignment for the two batched reciprocals)
        den12 = main.tile([36, NQ], F32R)
        recip12 = main.tile([36, NQ], F32R)
        drow = lambda h: h if h < 8 else 24 + h
        GRP = [4, 2, 4, 2, 4]
        RBB = dict(tag="rbb", bufs=2)

        def normalize_pair(jo):
            # one [128, NQ] tile: head 2jo's recip row on partitions 0-63,
            # head 2jo+1's on 64-127 -> single full-width multiply
            rb = main.tile([128, NQ], F32R, **RBB)
            hA, hB = 2 * jo, 2 * jo + 1
            _bcast_row(nc, dbc, rb[0:DH, :], recip12[drow(hA):drow(hA) + 1, :])
            _bcast_row(nc, dbc, rb[DH:128, :], recip12[drow(hB):drow(hB) + 1, :])
            nc.vector.tensor_mul(oT[:, jo, :], oT[:, jo, :], rb)

        with tc.tile_pool(name="psca", bufs=1, space="PSUM") as psca, \
             tc.tile_pool(name="pscb", bufs=1, space="PSUM") as pscb, \
             tc.tile_pool(name="pop", bufs=2, space="PSUM") as pop:
            for h in range(H):
                jo, pb = h // 2, (h % 2) * DH
                po = pop.tile([DH + 1, NQ], F32, tag="po")
                cursor = 0
                for gi, n in enumerate(GRP):
                    if gi % 2 == 0:
                        psc = psca.tile([128, 4 * NQ], F32, tag="sca")
                    else:
                        psc = pscb.tile([128, 2 * NQ], F32, tag="scb")
                    for i in range(n):
                        tp = cursor + i
                        kb = k_tiles[tp // 4][pb:pb + DH, jo,
                                              (tp % 4) * 128:(tp % 4 + 1) * 128]
                        nc.tensor.matmul(psc[:, i * NQ:(i + 1) * NQ],
                                         kb, qT[pb:pb + DH, jo, :],
                                         start=True, stop=True)
                    # exp shifted by -2 (softmax-invariant) to fit fp8e4 range
                    et = main.tile([128, 4 * NQ], FP8, **ET)
                    nc.scalar.activation(et[:, 0:n * NQ], psc[:, 0:n * NQ],
                                         AF.Exp, bias=neg2_c, scale=0.125)
                    for i in range(0, n, 2):
                        tpp = (cursor + i) // 2
                        nc.tensor.matmul(
                            po, v_tiles[tpp][:, :, h, 0:DH + 1],
                            et[:, i * NQ:(i + 2) * NQ].rearrange(
                                "p (two n) -> p two n", two=2),
                            start=(tpp == 0), stop=(tpp == 7),
                            perf_mode=DR)
                    cursor += n
                # stage unnormalized output + denominator row
                nc.vector.tensor_copy(oT[pb:pb + DH, jo, :], po[0:DH, :])
                ds = main.tile([1, NQ], F32R, **DST)
                nc.vector.tensor_copy(ds, po[DH:DH + 1, :])
                nc.sync.dma_start(den12[drow(h):drow(h) + 1, :], ds)
                if h == 7:
                    with nc.allow_low_precision(reason="softmax denom f32r"):
                        nc.vector.reciprocal(recip12[0:8, :], den12[0:8, :])
                if h == 9:
                    # normalize heads 0-7 while the last pairs compute
                    for j2 in range(4):
                        normalize_pair(j2)
            with nc.allow_low_precision(reason="softmax denom f32r"):
                nc.vector.reciprocal(recip12[32:36, :], den12[32:36, :])
            normalize_pair(4)
            normalize_pair(5)

        # ---- out-proj (+ residual), LN2, MLP
        ph2_mm = tc.tile_pool(name="pmm2", bufs=3, space="PSUM")
        ph2_st = tc.tile_pool(name="pstats2", bufs=1, space="PSUM")
        ph2_bc = tc.tile_pool(name="prbmb2", bufs=2, space="PSUM")
        psmm2 = ph2_mm.__enter__()
        stats2 = ph2_st.__enter__()
        rbmb2 = ph2_bc.__enter__()

        # out-proj (w_out pre-gated by gate_msa on host) + residual -> x2
        x2 = main.tile([128, DC, NQ], F32R)
        for half in range(2):
            wo = main.tile([128, DC, 384], BF16, name=f"wo{half}", **WQ)
            nc.sync.dma_start(wo, wout_r[:, :, half * 384:(half + 1) * 384])
            for sub in range(3):
                mo = half * 3 + sub
                ps = psmm2.tile([128, NQ], F32, tag="mm")
                for o in range(DC):
                    nc.tensor.matmul(
                        ps, wo[:, o, sub * 128:(sub + 1) * 128], oT[:, o, :],
                        start=(o == 0), stop=(o == DC - 1))
                nc.vector.tensor_add(x2[:, mo, :], ps, xskip[:, mo, :])

        # LN2 on x2 -> xhat2 (bf16), same folded scheme
        xb2 = main.tile([128, DC, NQ], BF16)
        for o in range(DC):
            nc.vector.tensor_copy(xb2[:, o, :], x2[:, o, :])
        ps_s2 = stats2.tile([33, NQ], F32, tag="st")
        sqs = []
        for o in range(DC):
            sq = main.tile([128, NQ], BF16, **SQ)
            nc.vector.tensor_mul(sq, xb2[:, o, :], xb2[:, o, :])
            sqs.append(sq)
        for o in range(DC):
            nc.tensor.matmul(ps_s2[0:1, :], ones_bf, xb2[:, o, :],
                             start=(o == 0), stop=(o == DC - 1))
            nc.tensor.matmul(ps_s2[32:33, :], ones_bf, sqs[o],
                             start=(o == 0), stop=(o == DC - 1))
        rstd2, negmr2 = _ln_rows(nc, rows, ps_s2[0:1, :], ps_s2[32:33, :],
                                 ln_mode)
        rb_ps = rbmb2.tile([128, NQ], F32, tag="bc")
        nc.tensor.matmul(rb_ps, ones_row, rstd2, start=True, stop=True)
        mb_ps = rbmb2.tile([128, NQ], F32, tag="bc")
        nc.tensor.matmul(mb_ps, ones_row, negmr2, start=True, stop=True)
        rb_sb = main.tile([128, NQ], BF16, **RBSB)
        nc.vector.tensor_copy(rb_sb, rb_ps)
        mb_sb = main.tile([128, NQ], BF16, **RBSB)
        nc.vector.tensor_copy(mb_sb, mb_ps)
        for o in range(DC):
            nc.vector.tensor_mul(xb2[:, o, :], xb2[:, o, :], rb_sb)
            nc.vector.tensor_add(xb2[:, o, :], xb2[:, o, :], mb_sb)

        # MLP1: hT = gelu_tanh(w1a @ xhat2 + cm1), 4 tiles [128, 6, 512]
        for wt in range(4, 6):
            w_t = main.tile([128, DC, NQ], BF16, name=f"w1_{wt}", **KTR)
            nc.sync.dma_start(w_t, wm1_r[:, :, wt * 512:(wt + 1) * 512])
            w1_tiles.append(w_t)
        h_tiles = [main.tile([128, DC, NQ], BF16, name=f"hT{i}", **BIG)
                   for i in range(4)]
        for wt in range(6):
            for sub in range(4):
                ho = wt * 4 + sub
                ps = psmm2.tile([128, NQ], F32, tag="mm")
                for o in range(DC):
                    nc.tensor.matmul(
                        ps, w1_tiles[wt][:, o, sub * 128:(sub + 1) * 128],
                        xb2[:, o, :],
                        start=(o == 0), stop=(o == DC - 1))
                _gelu_store(nc, main, ps, cm1_c[:, ho:ho + 1],
                            h_tiles[ho // 6][:, ho % 6, :], gelu_mode)

        # MLP2 (w_m2 pre-gated by gate_mlp) + residual, streamed out per mo
        for wt in range(6):
            w_t = main.tile([128, HC, 128], BF16, name=f"w2_{wt}", **KTR)
            nc.sync.dma_start(w_t, wm2_r[:, :, wt * 128:(wt + 1) * 128])
            ps = psmm2.tile([128, NQ], F32, tag="mm")
            for ko in range(HC):
                nc.tensor.matmul(
                    ps, w_t[:, ko, :], h_tiles[ko // 6][:, ko % 6, :],
                    start=(ko == 0), stop=(ko == HC - 1))
            tmp = main.tile([128, NQ], F32, tag="tmp", bufs=2)
            nc.scalar.activation(tmp, ps, AF.Identity,
                                 bias=gb2_c[:, wt:wt + 1], scale=1.0)
            nc.vector.tensor_add(x2[:, wt, :], tmp, x2[:, wt, :])
            nc.sync.dma_start(outT_r[:, wt, :], x2[:, wt, :])

        ph2_bc.__exit__(None, None, None)
        ph2_st.__exit__(None, None, None)
        ph2_mm.__exit__(None, None, None)


def _fix_module_for_walrus(nc):
    """Workarounds for this container's walrus build:
    (a) it rejects >1 sync-wait per instruction ("Too many sync wait
        commands") -> hoist extra waits onto NoOp carrier instructions;
    (b) it rejects custom Pool InstISA ("ISA wrong length") -> expand the
        tail EVENT_SEMAPHORE_RANGE_CLEAR into per-sem sem-sub-imm updates
        using the final values observed in earlier waits.
    """
    import bass_rust
    nid = [0]

    def carrier(engine, wait):
        nop = mybir.InstNoOp(name=f"wsplit_{nid[0]}", ins=[], outs=[])
        nid[0] += 1
        nop.engine = engine
        nop.sync_info = mybir.SyncInfo(on_wait=[wait], on_update=[])
        return nop

    for f in nc.m.functions:
        new_blocks = []
        for bb in f.blocks:
            sem_final = {}
            out = []
            for inst in bb.instructions:
                si = inst.sync_info
                if si is not None:
                    for w in si.on_wait:
                        if w.sync_type == "semaphore" and w.wait_mode == "sem-ge-imm":
                            sem_final[w.id] = max(sem_final.get(w.id, 0),
                                                  w.wait_value)
                if (type(inst).__name__ == "InstISA"
                        and getattr(inst, "op_name", "") ==
                        "EVENT_SEMAPHORE_RANGE_CLEAR"):
                    ad = inst.ant_dict
                    lo, hi = ad["range_first"], ad["range_last"]
                    waits = list(si.on_wait) if si else []
                    for w in waits:
                        out.append(carrier(inst.engine, w))
                    for sem_id in range(lo, hi + 1):
                        v = sem_final.get(sem_id, 0)
                        if v == 0:
                            continue
                        ev = mybir.InstEventSemaphore(
                            name=f"semclr_{nid[0]}", ins=[], outs=[])
                        nid[0] += 1
                        ev.engine = inst.engine
                        ev.sync_info = mybir.SyncInfo(
                            on_wait=[],
                            on_update=[mybir.SyncUpdate(
                                sync_type="semaphore", id=sem_id,
                                ant_name=f"clr{sem_id}",
                                update_mode="sem-sub-imm", update_value=v,
                                update_reg=None)])
                        out.append(ev)
                    continue
                if type(inst).__name__ == "InstISA":
                    raise RuntimeError(
                        f"unsupported InstISA {getattr(inst, 'op_name', '?')}")
                waits = list(si.on_wait) if si else []
                if len(waits) > 1:
                    for w in waits[:-1]:
                        out.append(carrier(inst.engine, w))
                    inst.sync_info = mybir.SyncInfo(
                        on_wait=waits[-1:], on_update=list(si.on_update))
                out.append(inst)
            nbb = bass_rust.BasicBlock(name=bb.name, instructions=out)
            for attr in ("IsExit", "IsLoopEntry", "IsPredicated"):
                try:
                    setattr(nbb, attr, getattr(bb, attr))
                except Exception:
                    pass
            new_blocks.append(nbb)
        f.blocks = new_blocks
    return nc


def _build_nc(gelu_mode="fused", prec="bf16", ln_mode="lnexp"):
    nc = bass.Bass(
        "TRN2", target_bir_lowering=False, debug=False, enable_asserts=False,
        num_devices=8,
    )
    shapes = {
        "xT": ([D, S], FP8),
        "xskipT": ([D, NQ], F32R),
        "cols_c": ([128, 44], F32),
        "ones_r": ([1, 128], F32R),
        "w_qkvT": ([D, 3 * D], FP8),
        "w_outT": ([D, D], BF16),
        "w_m1T": ([D, HID], BF16),
        "w_m2T": ([HID, D], BF16),
    }
    dram = {k: nc.dram_tensor(k, shp, dt, kind="ExternalInput")
            for k, (shp, dt) in shapes.items()}
    dram["outT"] = nc.dram_tensor("outT", [D, NQ], F32R, kind="ExternalOutput")
    with tile.TileContext(nc) as tc:
        _body(tc, dram, gelu_mode, ln_mode)
    return nc


def _ensure_fixed(nc):
    if not getattr(nc, "_walrus_fixed", False):
        _fix_module_for_walrus(nc)
        nc._walrus_fixed = True
    return nc


_NC_CACHE = {}


def _get_nc(gelu_mode="fused", prec="bf16", ln_mode="lnexp"):
    key = (gelu_mode, prec, ln_mode)
    if key not in _NC_CACHE:
        _NC_CACHE[key] = _build_nc(gelu_mode, prec, ln_mode)
    return _NC_CACHE[key]


def _colpack(v, nch):
    """[nch*128] vector -> [128, nch] column-packed (col jo = v[jo*128+p])."""
    return np.ascontiguousarray(np.asarray(v, np.float32).reshape(nch, 128).T)


def make_in_maps(inputs, prec="bf16"):
    import ml_dtypes
    bf16 = ml_dtypes.bfloat16
    fp8 = ml_dtypes.float8_e4m3
    x = np.asarray(inputs["x"], np.float32)
    c = np.asarray(inputs["c"], np.float32)
    w_ada = np.asarray(inputs["w_ada"], np.float32)
    b_ada = np.asarray(inputs["b_ada"], np.float32)
    w_qkv = np.asarray(inputs["w_qkv"], np.float32)
    w_out = np.asarray(inputs["w_out"], np.float32)
    w_m1 = np.asarray(inputs["w_mlp1"], np.float32)
    b_m1 = np.asarray(inputs["b_mlp1"], np.float32)
    w_m2 = np.asarray(inputs["w_mlp2"], np.float32)
    b_m2 = np.asarray(inputs["b_mlp2"], np.float32)
    n1 = np.asarray(inputs["norm1_w"], np.float32)
    n2 = np.asarray(inputs["norm2_w"], np.float32)

    ada = c @ w_ada.T + b_ada                      # (2, 4608)
    tr = lambda w: np.ascontiguousarray(w.T.astype(bf16))
    in_maps = [None] * 8
    for b in range(2):
        sh1, sc1, g1, sh2, sc2, g2 = ada[b].reshape(6, D)
        a1 = (1.0 + sc1) * n1
        a2 = (1.0 + sc2) * n2
        cqkv = w_qkv @ sh1                         # (2304,)
        cv = cqkv[2 * D:3 * D]
        cm1 = w_m1 @ sh2 + b_m1                    # (3072,)
        cols = np.concatenate([
            _colpack(cqkv[0:D], 6),                # q bias
            _colpack(cqkv[D:2 * D], 6),            # k bias
            _colpack(cm1, 24),                     # mlp1 bias (gelu)
            _colpack(g2 * b_m2, 6),                # gated mlp2 bias
            np.ones((128, 1), np.float32),         # ones column
            np.full((128, 1), -2.0, np.float32),   # exp shift (fp8 range)
        ], axis=1)
        xskip_add = (g1 * (w_out @ cv))[:, None]   # V-shift const, post-gate
        base = {
            "cols_c": np.ascontiguousarray(cols, dtype=np.float32),
            "ones_r": np.ones((1, 128), np.float32),
            "w_qkvT": np.ascontiguousarray(
                (w_qkv * a1[None, :] * W8SCALE).T.astype(fp8)),
            "w_outT": tr(w_out * g1[:, None]),
            "w_m1T": tr(w_m1 * a2[None, :]),
            "w_m2T": tr(w_m2 * g2[:, None]),
        }
        for k in range(4):
            xb = np.roll(x[b], -NQ * k, axis=0)    # my queries first
            m = dict(base)
            m["xT"] = np.ascontiguousarray(xb.T.astype(fp8))
            m["xskipT"] = np.ascontiguousarray(xb[0:NQ].T + xskip_add)
            in_maps[b * 4 + k] = m
    return in_maps


def assemble_output(results):
    out = np.empty((2, S, D), np.float32)
    for core in range(8):
        b, k = core // 4, core % 4
        out[b, NQ * k:NQ * (k + 1)] = results[core]["outT"].T
    return out


def kernel(**inputs):
    nc = _ensure_fixed(_get_nc())
    in_maps = make_in_maps(inputs)
    res = run_bass_kernel_spmd(nc, in_maps, core_ids=list(range(8)))
    return assemble_output(res.results)


if __name__ == "__main__":
    _get_nc()
    print("build ok")
